# revision 77
# baseline (speedup 1.0000x reference)
"""Trainium2 Bass kernel for nn_BaselineAttnDecoder.

Data-parallel over 8 NeuronCores: each core handles 160 decode rows
(16 images x 10 rounds). All weights replicated.

Layout: transposed compute — activations live as [feature, batch] tiles
(feature on partitions, batch=160 on the matmul moving dim), weights as
stationary lhsT blocks [in-tile, out-tile]. GRU gates come out of PSUM
already transposed, so the hidden state is never transposed. Gate
biases ride a constant-1.0 row (feature 320) appended to the padded
embedding table, so r+z share one batched tanh per j-tile and the n
tanh needs no bias; gates use tanh-form sigmoids (sigmoid(x) =
(tanh(x/2)+1)/2, 0.5 factors folded into host-side weights) so only
one activation table is ever loaded. The image attention value matrix
is folded through the GRU/out weights once per core
(WivT = i_value @ W_ic^T), so the image context is never materialized;
its softmax uses a multiplicative 0/1 mask fused after an exp read
straight from PSUM. The question context uses a flipped diag-trick
(lhsT = q_value b-layout, rhs = diag(qw)) built as per-l strips on
DVE/Pool so the qc matmuls start as soon as the first strip lands.
Softmax skips max-subtraction (score ranges verified tiny); the
question mask rides an extra 51st column of the score product.

Final-step argmax: the embedding table is prefetched into SBUF chunk-
by-chunk during decoder steps; logits are scanned per 512-chunk with
top-8 max/max_index directly on PSUM f32 (exact logits, first-
occurrence ties, chunk-major), then a stacked global top-1 and one-hot
index extraction replace the old full-width bf16 scans + f32 rescore.
"""
import numpy as np
import ml_dtypes

import concourse.bass as bass
import concourse.bacc as bacc
import concourse.mybir as mybir
import concourse.tile as tile
from concourse.masks import make_identity

F32 = mybir.dt.float32
BF16 = mybir.dt.float16  # 16-bit compute dtype (fp16: 10-bit mantissa)
U32 = mybir.dt.uint32
U16 = mybir.dt.uint16
AF = mybir.ActivationFunctionType
ALU = mybir.AluOpType
AX = mybir.AxisListType

D, H, V, K = 300, 512, 8835, 50
K1 = K + 1  # score contraction + mask column
L, MAX_LEN, ROUNDS = 20, 21, 10
BS = 160
NCORES = 8
PBS = [128, 32]
BOFF = [0, 128]
IL = 256
VP = 18 * 512
NEG = -30.0  # mask offset on score scale (|scores| < 4)
XROWS = [128, 128, 44]
XROWS_X = [128, 128, 65]  # row 320 = const 1.0 (gate biases ride matmul)


def bcast_mid(ap, reps):
    return bass.AP(tensor=ap.tensor, offset=ap.offset,
                   ap=[ap.ap[0], [0, reps], ap.ap[1]])


def bcast_in(ap, reps):
    return bass.AP(tensor=ap.tensor, offset=ap.offset,
                   ap=[ap.ap[0], ap.ap[1], [0, reps]])


def build_nc(n_enc=None):
    """n_enc[t]: number of rows (sorted by question length, descending)
    still active at encoder step t. Baked into the program; derived from
    the actual ques_lens at first kernel() call."""
    if n_enc is None:
        n_enc = [BS] * L
    n0 = [min(128, n) for n in n_enc]           # bt0 active rows at step l
    n1 = [max(0, n - 128) for n in n_enc]       # bt1 active rows
    nc = bacc.Bacc()

    def din(name, shape, dt):
        return nc.dram_tensor(name, shape, dt, kind="ExternalInput")

    w_gi = din("w_gi", [128, 11, 3 * H], BF16)
    w_gh = din("w_gh", [128, 4, 3 * H], BF16)   # n-cols pre-scaled by 0.5
    bnrow = din("bnrow", [1, H], BF16)          # 0.5*bhh_n
    w_egi = din("w_egi", [128, 3, 3 * H], BF16)
    w_egh = din("w_egh", [128, 4, 3 * H], BF16)
    ebnrow = din("ebnrow", [1, H], BF16)
    w_out = din("w_out", [128, 12, D], BF16)
    outb_c = din("outb_c", [128, 3], F32)
    w_qk = din("w_qk", [128, 4, K], BF16)
    qkb = din("qkb", [1, K], BF16)
    w_qv = din("w_qv", [128, 4, H], BF16)
    w_ak = din("w_ak", [128, 4, K], BF16)
    akb = din("akb", [1, K], BF16)
    akb_c = din("akb_c", [128, 1], F32)
    w_ik = din("w_ik", [128, 2, K], BF16)
    ikb_c = din("ikb_c", [128, 1], F32)
    w_iv = din("w_iv", [128, 2, H], BF16)
    img_t = din("img_t", [128, 2, IL], BF16)
    emb_bf = din("emb_bf", [V, 384], BF16)
    embt_bf = din("embt_bf", [128, 3, VP], BF16)
    q_idx = din("q_idx", [128, 2 * L], U32)
    a_idx = din("a_idx", [128, 2 * L], U32)
    qmask = din("qmask", [128, 2, L], BF16)
    qmask1 = din("qmask1", [128, 5], BF16)
    ie_mask = din("ie_mask", [128, 2, IL], BF16)
    qoff = din("qoff", [128, 144], F32)

    out_o = nc.dram_tensor("out_o", [MAX_LEN, 3, 128, BS], F32,
                           kind="ExternalOutput")

    with tile.TileContext(nc) as tc:
        with (
            tc.tile_pool(name="cw", bufs=1) as cw,
            tc.tile_pool(name="pers", bufs=1) as pers,
            tc.tile_pool(name="wk", bufs=2) as wk,
            tc.tile_pool(name="st", bufs=2) as st,
            tc.tile_pool(name="pg", bufs=3, space="PSUM") as pg,
            tc.tile_pool(name="pm", bufs=2, space="PSUM") as pm,
            tc.tile_pool(name="pq", bufs=1, space="PSUM") as pq,
            tc.tile_pool(name="pt", bufs=2, space="PSUM") as pt,
        ):
            def load(pool, t, dt):
                s = pool.tile(list(t.shape), dt, name=t.name + "_sb")
                nc.sync.dma_start(s[:], t[:])
                return s

            # load order = need order: gather indices and encoder weights
            # first so the encoder starts while the big tables stream in
            from contextlib import ExitStack as _ES
            qp_stack = _ES()
            qp = qp_stack.enter_context(tc.tile_pool(name="qp", bufs=1))
            s_qidx = load(cw, q_idx, U32)
            s_egi = load(qp, w_egi, BF16)
            s_egh = load(qp, w_egh, BF16)
            s_imgt = load(cw, img_t, BF16)
            s_iv = load(cw, w_iv, BF16)
            s_ik = load(cw, w_ik, BF16)
            s_ikb = load(cw, ikb_c, F32)
            s_qk = load(cw, w_qk, BF16)
            s_qv = load(cw, w_qv, BF16)
            s_qkb = load(cw, qkb, BF16)
            s_ebnrow = load(cw, ebnrow, BF16)
            s_aidx = load(cw, a_idx, U32)
            s_gi = load(cw, w_gi, BF16)
            s_gh = load(cw, w_gh, BF16)
            s_out = load(cw, w_out, BF16)
            s_ak = load(cw, w_ak, BF16)
            s_bnrow = load(cw, bnrow, BF16)
            s_outb = load(cw, outb_c, F32)
            s_akb = load(cw, akb, BF16)
            s_akbc = load(cw, akb_c, F32)
            s_iem = load(cw, ie_mask, BF16)

            ident_bf = cw.tile([128, 128], BF16)
            make_identity(nc, ident_bf[:])
            ones_bf = cw.tile([1, 128], BF16)
            nc.vector.memset(ones_bf[:], 1.0)
            onesb = cw.tile([1, BS], BF16)
            nc.vector.memset(onesb[:], 1.0)
            sid4 = cw.tile([128, 32], BF16)
            for g4 in range(4):
                nc.vector.tensor_copy(sid4[32 * g4:32 * (g4 + 1), :],
                                      ident_bf[0:32, 0:32])
            iota144 = cw.tile([128, 144], F32)
            nc.gpsimd.iota(iota144[:], pattern=[[1, 144]], base=0,
                           channel_multiplier=0,
                           allow_small_or_imprecise_dtypes=True)
            off144 = load(cw, qoff, F32)

            # sid4 transposed: sid4T[r, p] = 1 iff p % 32 == r (replicate
            # a [32]-vector across the 4 partition groups via matmul)
            sid4T = cw.tile([32, 128], BF16)
            ps4t = pt.tile([128, 128], BF16, tag="trp", name="ps4t")
            nc.tensor.transpose(ps4t[0:32, 0:128], sid4[:, 0:32],
                                ident_bf[:, :])
            nc.vector.tensor_copy(sid4T[:, :], ps4t[0:32, 0:128])

            hT = pers.tile([128, 4, BS], BF16)
            qk_b0 = pers.tile([128, L, K1], BF16)
            qk_b1 = pers.tile([128, L, K1], BF16)
            qv_b0 = pers.tile([128, L, H], BF16)
            qv_p1 = pers.tile([128, 5, H], BF16)
            ikt = pers.tile([128, IL], BF16)
            ivT = pers.tile([128, 4, IL], BF16)
            wivT = pers.tile([128, 2, 3 * H], BF16)
            woivT = pers.tile([128, 2, D], BF16)
            dec20 = pers.tile([128, 3, BS], BF16)
            a_bf = pers.tile([128, 2, K1], BF16)

            nc.vector.memset(hT[:], 0.0)
            nc.vector.memset(a_bf[:, 0, K:K1], 1.0)
            nc.vector.memset(a_bf[:, 1, K:K1], 1.0)
            # bt1 projections are skipped for steps past bt1's max length;
            # zero-init so skipped slots contribute exactly nothing
            nc.vector.memset(qk_b1[:], 0.0)
            nc.vector.memset(qv_p1[:], 0.0)
            # question mask -> 51st score column (once)
            nc.sync.dma_start(qk_b0[:, 0:L, K:K1], qmask[:, 0, :])
            nc.sync.dma_start(qk_b1[:, 0:L, K:K1], qmask[:, 1, :])

            def tr(dst_sb_ap, src_sb_ap, pb, w, eng=None):
                p = pt.tile([128, 128], BF16, tag="trp", name="p")
                nc.tensor.transpose(p[:w, :pb], src_sb_ap, ident_bf[:pb, :pb])
                if eng is nc.scalar:
                    nc.scalar.copy(dst_sb_ap, p[:w, :pb])
                else:
                    (eng or nc.vector).tensor_copy(dst_sb_ap, p[:w, :pb])

            def fetch_gather(idx_sb, t):
                gs = []
                for c, (pb, off) in enumerate(zip(PBS, BOFF)):
                    g = wk.tile([128, 384], BF16, tag="gath", bufs=5,
                                name="g")
                    nc.gpsimd.indirect_dma_start(
                        out=g[:pb], out_offset=None, in_=emb_bf[:],
                        in_offset=bass.IndirectOffsetOnAxis(
                            ap=idx_sb[:pb, 2 * t + c:2 * t + c + 1], axis=0))
                    gs.append(g)
                return gs

            def fetch_tr(gs, eng=None):
                xt = wk.tile([128, 3, BS], BF16, tag="xt", bufs=6, name="xt")
                for c, (pb, off) in enumerate(zip(PBS, BOFF)):
                    g = gs[c]
                    p3 = pt.tile([128, 3, 128], BF16, tag="trp", name="p3")
                    for kt in range(3):
                        nc.tensor.transpose(p3[:, kt, :pb],
                                            g[:pb, kt * 128:(kt + 1) * 128],
                                            ident_bf[:pb, :pb])
                    dst = xt[0:128, :, off:off + pb]
                    if eng is nc.scalar:
                        nc.scalar.copy(dst, p3[:, :, :pb])
                    else:
                        (eng or nc.vector).tensor_copy(dst, p3[:, :, :pb])
                return xt

            def fetch_x(idx_sb, t, eng=None):
                return fetch_tr(fetch_gather(idx_sb, t), eng=eng)

            def emit_acc(ps_ap, pairs):
                n = len(pairs)
                for i, (lh, rh) in enumerate(pairs):
                    nc.tensor.matmul(ps_ap, lh, rh, start=(i == 0),
                                     stop=(i == n - 1))

            # Transposed GRU step. xk: 3 rhs APs for dec_in k-tiles.
            # extra: list of (w_tensor, rhs_tensor, k_offset, n_k).
            def emit_bn(wh, bn_row, gbn1, gbn2, j, nb=BS):
                gb = (gbn1[:, j * BS:j * BS + nb] if j < 3
                      else gbn2[:, 0:nb])
                pairs = [(wh[:, kt, 1024 + j * 128:1024 + (j + 1) * 128],
                          hT[:, kt, :nb]) for kt in range(4)]
                pairs.append((bn_row[0:1, j * 128:(j + 1) * 128],
                              onesb[:, :nb]))
                emit_acc(gb, pairs)
                return gb

            def gru_t(xk, wx, wh, bn_row, extra, nb=BS,
                      x_last=False, bns=None):
                # gate biases ride the x matmuls (constant-1 row at
                # feature 300), so r+z share one batched tanh per j
                if bns is None:
                    gbn1 = pm.tile([128, 512], F32, tag="mix", name="gbn1")
                    gbn2 = pm.tile([128, 512], F32, tag="mix", name="gbn2")
                    gbs = [emit_bn(wh, bn_row, gbn1, gbn2, j, nb=nb)
                           for j in range(4)]
                else:
                    gbn1, gbn2, gbs = bns
                accs = []
                for j in range(4):
                    g3 = pg.tile([128, 3, BS], F32, tag="g3", name="g3")
                    gb = gbs[j]
                    for gi_, g in enumerate((j, 4 + j, 8 + j)):  # r, z, n
                        mc = slice(g * 128, (g + 1) * 128)
                        xp = [(wx[:kr, kt, mc], xk[kt][:kr, :nb])
                              for kt, kr in enumerate(XROWS_X)]
                        pairs = [] if x_last else list(xp)
                        if g < 8:
                            pairs += [(wh[:, kt, mc], hT[:, kt, :nb])
                                      for kt in range(4)]
                        for (wt, rt, koff, nk) in extra:
                            pairs += [(wt[:, koff + kt, mc], rt[:, kt, :nb])
                                      for kt in range(nk)]
                        if x_last:
                            pairs += xp
                        emit_acc(g3[:, gi_, :nb], pairs)
                    accs.append((g3, gb))
                for j in range(4):
                    g3, gb = accs[j]
                    trz = st.tile([128, 2, BS], BF16, tag="trz", bufs=4,
                                  name="trz")
                    nc.scalar.activation(trz[:, :, :nb], g3[:, 0:2, :nb],
                                         AF.Tanh, scale=0.5)
                    # PSUM operands -> DVE only (GPSIMD cannot touch PSUM)
                    u = st.tile([128, BS], F32, tag="u", bufs=4, name="u")
                    nc.vector.scalar_tensor_tensor(u[:, :nb],
                                                   trz[:, 0, :nb],
                                                   1.0, gb, op0=ALU.add,
                                                   op1=ALU.mult)
                    nc.vector.tensor_add(u[:, :nb], u[:, :nb],
                                         g3[:, 2, :nb])
                    n_ = st.tile([128, BS], BF16, tag="n_", bufs=4, name="n_")
                    nc.scalar.activation(n_[:, :nb], u[:, :nb], AF.Tanh)
                    # h' = n + z(h-n) with z = (tzz+1)/2:
                    #   d = h-n; e = tzz*d + d = 2z*d; h' = 0.5*e + n
                    # (Pool supports only plain tensor-tensor ops on HW)
                    d = st.tile([128, BS], BF16, tag="d", bufs=4, name="d")
                    e = st.tile([128, BS], BF16, tag="e", bufs=4, name="e")
                    nc.gpsimd.tensor_sub(d[:, :nb], hT[:, j, :nb],
                                         n_[:, :nb])
                    nc.gpsimd.tensor_mul(e[:, :nb], trz[:, 1, :nb],
                                         d[:, :nb])
                    nc.gpsimd.tensor_add(e[:, :nb], e[:, :nb], d[:, :nb])
                    nc.vector.scalar_tensor_tensor(hT[:, j, :nb], e[:, :nb],
                                                   0.5, n_[:, :nb],
                                                   op0=ALU.mult, op1=ALU.add)
                return gbn2

            # ---------- encoder ----------
            def enc_proj(t, gbn2):
                # q_key / q_value projections for step t (b-layout)
                for bt in range(2):
                    if bt == 1 and n1[t] == 0:
                        continue
                    pb, off = PBS[bt], BOFF[bt]
                    sl = slice(off, off + pb)
                    psk = gbn2[:, BS + bt * 64:BS + bt * 64 + K]
                    pairs = [(hT[:, kt, sl], s_qk[:, kt, :])
                             for kt in range(4)]
                    pairs.append((ones_bf[:, :pb], s_qkb[:]))
                    emit_acc(psk[:pb], pairs)
                    if bt == 0:
                        nc.scalar.copy(qk_b0[:pb, t, 0:K], psk[:pb])
                    else:
                        nc.scalar.copy(qk_b1[:pb, t, 0:K], psk[:pb])
                    psv = pq.tile([128, 512], F32, tag="qc", name="psvq")
                    emit_acc(psv[:pb, :],
                             [(hT[:, kt, sl], s_qv[:, kt, :])
                              for kt in range(4)])
                    if bt == 0:
                        nc.scalar.copy(qv_b0[:pb, t, :], psv[:pb, :])
                    else:
                        g4 = t % 4
                        nc.scalar.copy(qv_p1[32 * g4:32 * (g4 + 1),
                                             t // 4, :], psv[:pb, :])

            if True:
                # image projections (need only the early loads)
                for hc in range(4):
                    psv = pq.tile([128, 512], F32, tag="qc", name="psv")
                    emit_acc(psv[:, 0:IL],
                             [(s_iv[:, kt, hc * 128:(hc + 1) * 128],
                               s_imgt[:, kt, :]) for kt in range(2)])
                    nc.scalar.copy(ivT[:, hc, :], psv[:, 0:IL])
                psik = pq.tile([128, 512], F32, tag="qc", name="psik")
                emit_acc(psik[:K, 0:IL], [(s_ik[:, kt, :], s_imgt[:, kt, :])
                                          for kt in range(2)])
                nc.vector.tensor_scalar_add(ikt[:K, :], psik[:K, 0:IL],
                                            s_ikb[:K, :])
                exts = [fetch_x(s_qidx, 0), fetch_x(s_qidx, 1)]
                prev_gbn2 = None
                for t in range(L):
                    if t + 2 < L:
                        pend = fetch_gather(s_qidx, t + 2)
                    if t >= 1:
                        enc_proj(t - 1, prev_gbn2)
                    xt = exts[t]
                    xk = [xt[:, 0, :], xt[:, 1, :], xt[:, 2, :]]
                    prev_gbn2 = gru_t(xk, s_egi, s_egh, s_ebnrow,
                                      [], nb=n_enc[t])
                    if t + 2 < L:
                        exts.append(fetch_tr(
                            pend, eng=nc.scalar if t % 2 else None))
                enc_proj(L - 1, prev_gbn2)

            qp_stack.close()
            nc.vector.memset(hT[:], 0.0)

            # image-value weight folds (need the big late weight loads;
            # consumed only by the decoder)
            for vt in range(2):
                for ncx in range(3):
                    psw = pq.tile([128, 512], F32, tag="qc", name="psw")
                    emit_acc(psw[:, :],
                             [(ivT[:, hc, vt * 128:(vt + 1) * 128],
                               s_gi[:, 7 + hc, ncx * 512:(ncx + 1) * 512])
                              for hc in range(4)])
                    nc.scalar.copy(wivT[:, vt, ncx * 512:(ncx + 1) * 512],
                                   psw[:, :])
                psw = pq.tile([128, 512], F32, tag="qc", name="psw2")
                emit_acc(psw[:, 0:D],
                         [(ivT[:, hc, vt * 128:(vt + 1) * 128],
                           s_out[:, 8 + hc, :]) for hc in range(4)])
                nc.scalar.copy(woivT[:, vt, :], psw[:, 0:D])

            # ---------- decoder ----------
            with tc.tile_pool(name="lg", bufs=1) as lg:
                o19T = lg.tile([128, 3, BS], BF16)
                nc.vector.memset(o19T[:], 0.0)
                nc.vector.memset(o19T[64:65, 2, :], 1.0)
                # full embedding table prefetched chunk-by-chunk during
                # decoder steps; logits never wait on HBM
                s_embt = lg.tile([128, 3, VP], BF16)
                mxs0 = lg.tile([128, 144], F32)
                ixs0 = lg.tile([128, 144], U32)
                mxs1 = lg.tile([32, 144], F32)
                ixs1 = lg.tile([32, 144], U32)
                mxs = [mxs0, mxs1]
                ixs = [ixs0, ixs1]

                dxts = [fetch_x(s_aidx, 0, eng=nc.scalar),
                        fetch_x(s_aidx, 1, eng=nc.scalar)]
                dgaths = {2: fetch_gather(s_aidx, 2)}

                def out_proj(t, qcT, iwT):
                    # output bias is added on the host; PSUM goes straight
                    # to DRAM. Step 19 additionally materializes the biased
                    # output in SBUF (transposed) for the logits path.
                    for mt in range(3):
                        mw = XROWS[mt]
                        mc = slice(mt * 128, mt * 128 + mw)
                        pso = pq.tile([128, 512], F32, tag="qc", name="pso")
                        # ctx terms first: they are ready before the gru
                        # elementwise finishes; hT terms stagger per j
                        pairs = [(s_out[:, 4 + kt, mc], qcT[:, kt, :])
                                 for kt in range(4)]
                        pairs += [(woivT[:, kt, mc], iwT[:, kt, :])
                                  for kt in range(2)]
                        pairs += [(s_out[:, kt, mc], hT[:, kt, :])
                                  for kt in range(4)]
                        emit_acc(pso[:mw, 0:BS], pairs)
                        osb = st.tile([128, BS], F32, tag="osb", bufs=3,
                                      name="osb")
                        nc.scalar.add(osb[:mw, :], pso[:mw, 0:BS],
                                      s_outb[:mw, mt:mt + 1])
                        nc.sync.dma_start(out_o[t, mt, 0:mw, :], osb[:mw])
                        if t == MAX_LEN - 2:
                            nc.scalar.copy(o19T[:mw, mt, :], osb[:mw])

                def argmax_reembed():
                    # Per-chunk top-8 scans directly on PSUM f32 (exact
                    # logits; no bf16 round-trip and no rescore needed).
                    # Chunk-major stacking keeps first-occurrence argmax
                    # semantics.
                    for nci in range(18):
                        ncw = 512 if nci < 17 else V - 17 * 512
                        for bt in range(2):
                            pb, off = PBS[bt], BOFF[bt]
                            psl = pg.tile([128, 512], F32, tag="g3",
                                          name="psl")
                            pairs = []
                            for kt in range(3):
                                nr = 128 if kt < 2 else 65
                                pairs.append((o19T[:nr, kt, off:off + pb],
                                              s_embt[:nr, kt,
                                                     nci * 512:
                                                     nci * 512 + ncw]))
                            emit_acc(psl[:pb, 0:ncw], pairs)
                            nc.vector.max(mxs[bt][:pb, 8 * nci:8 * nci + 8],
                                          psl[:pb, 0:ncw])
                            nc.vector.max_index(
                                ixs[bt][:pb, 8 * nci:8 * nci + 8],
                                mxs[bt][:pb, 8 * nci:8 * nci + 8],
                                psl[:pb, 0:ncw])
                    for bt in range(2):
                        pb, off = PBS[bt], BOFF[bt]
                        g8 = st.tile([128, 8], F32, name="g8")
                        nc.vector.max(g8[:pb], mxs[bt][:pb])
                        p8 = st.tile([128, 8], U32, name="p8")
                        nc.vector.max_index(p8[:pb], g8[:pb], mxs[bt][:pb])
                        p0f = st.tile([128, 1], F32, name="p0f")
                        nc.vector.tensor_copy(p0f[:pb], p8[:pb, 0:1])
                        ixf = st.tile([128, 144], F32, tag="ixf", name="ixf")
                        nc.vector.tensor_copy(ixf[:pb], ixs[bt][:pb])
                        nc.vector.tensor_add(ixf[:pb], ixf[:pb],
                                             off144[:pb])
                        oh = st.tile([128, 144], F32, tag="oh", name="oh")
                        nc.vector.tensor_scalar(out=oh[:pb],
                                                in0=iota144[:pb],
                                                scalar1=p0f[:pb],
                                                scalar2=None,
                                                op0=ALU.is_equal)
                        nc.vector.tensor_mul(oh[:pb], oh[:pb], ixf[:pb])
                        vsum = st.tile([128, 1], F32, name="vsum")
                        nc.vector.tensor_reduce(vsum[:pb], oh[:pb],
                                                axis=AX.X, op=ALU.add)
                        vidx = st.tile([128, 1], U32, name="vidx")
                        nc.vector.tensor_copy(vidx[:pb], vsum[:pb])
                        gm = wk.tile([128, 384], BF16, tag="gath", bufs=5,
                                     name="gm")
                        nc.gpsimd.indirect_dma_start(
                            out=gm[:pb], out_offset=None, in_=emb_bf[:],
                            in_offset=bass.IndirectOffsetOnAxis(
                                ap=vidx[:pb, 0:1], axis=0))
                        for kt in range(3):
                            w = XROWS_X[kt]
                            tr(dec20[:w, kt, off:off + pb],
                               gm[:pb, kt * 128:kt * 128 + w], pb, w)

                for t in range(MAX_LEN):
                    if 2 <= t < 20:
                        c = t - 2
                        nc.sync.dma_start(
                            s_embt[:, :, c * 512:(c + 1) * 512],
                            embt_bf[:, :, c * 512:(c + 1) * 512])
                    if t + 3 < L:
                        dgaths[t + 3] = fetch_gather(s_aidx, t + 3)
                    # --- attention scores (b-layout) + aT (transposed) ---
                    att = pm.tile([128, 512], F32, tag="mix", name="att")
                    psa = att[:, 0:K]
                    pairs = [(hT[:, kt, 0:128], s_ak[:, kt, :])
                             for kt in range(4)]
                    pairs.append((ones_bf[:, 0:128], s_akb[:]))
                    emit_acc(psa[:128], pairs)
                    nc.scalar.copy(a_bf[:128, 0, 0:K], psa[:128])
                    psa1 = att[0:32, K:2 * K]
                    pairs = [(hT[:, kt, 128:BS], s_ak[:, kt, :])
                             for kt in range(4)]
                    pairs.append((ones_bf[:, 0:32], s_akb[:]))
                    emit_acc(psa1, pairs)
                    nc.scalar.copy(a_bf[0:32, 1, 0:K], psa1)
                    aTp = att[:, 2 * K:2 * K + BS]
                    emit_acc(aTp[:K, :], [(s_ak[:, kt, 0:K], hT[:, kt, :])
                                          for kt in range(4)])
                    aT = st.tile([128, BS], BF16, tag="aT", name="aT")
                    nc.scalar.add(aT[:K, :], aTp[:K, :], s_akbc[:K, :])

                    # --- image attention (early: only needs aT; its psum
                    # slot is free before the qc chain wants it) ---
                    iwT = st.tile([128, 2, BS], BF16, tag="iwT", name="iwT")
                    psi2 = pq.tile([128, 512], F32, tag="qc", name="psi2")
                    iwbs = []
                    for bt in range(2):
                        pb, off = PBS[bt], BOFF[bt]
                        sl = slice(off, off + pb)
                        psi = psi2[:, bt * IL:(bt + 1) * IL]
                        nc.tensor.matmul(psi[:pb], aT[:K, sl], ikt[:K, :],
                                         start=True, stop=True)
                    # --- bn accumulations (h-only; PE window fill) ---
                    gbn1 = pm.tile([128, 512], F32, tag="mix", name="gbn1")
                    gbn2 = pm.tile([128, 512], F32, tag="mix", name="gbn2")
                    gbs = [emit_bn(s_gh, s_bnrow, gbn1, gbn2, j)
                           for j in range(4)]

                    # --- question softmax (critical chain for qc); bt0
                    # on DVE, bt1 packed 4x so its ops are tiny ---
                    prod = wk.tile([128, L, K1], BF16, tag="prod",
                                   bufs=2, name="prod")
                    hl = L // 2
                    nc.vector.tensor_mul(prod[:, :], qk_b0[:, :],
                                         bcast_mid(a_bf[:, 0, :], L))
                    qe = st.tile([128, L], F32, name="qe")
                    nc.vector.tensor_reduce(qe[:, 0:hl], prod[:, 0:hl],
                                            axis=AX.X, op=ALU.add)
                    nc.vector.tensor_reduce(qe[:, hl:L], prod[:, hl:L],
                                            axis=AX.X, op=ALU.add)
                    ew0 = st.tile([128, L], F32, name="ew0")
                    ssum = st.tile([128, 1], F32, name="ssum")
                    nc.scalar.activation(ew0[:, :], qe[:, :], AF.Exp,
                                         scale=1.0, accum_out=ssum[:, :])
                    rs0 = st.tile([128, 1], F32, name="rs0")
                    nc.vector.reciprocal(rs0[:, :], ssum[:, :])

                    # bt1 softmax: reduce split into quarters so it never
                    # blocks the bt0 chain's recip/dg on the DVE stream
                    prod1 = wk.tile([128, L, K1], BF16, tag="prod1",
                                    bufs=1, name="prod1")
                    nc.gpsimd.tensor_mul(prod1[0:32, :], qk_b1[0:32, :],
                                         bcast_mid(a_bf[0:32, 1, :], L))
                    qe1 = st.tile([128, L], F32, name="qe1")
                    nc.vector.tensor_reduce(qe1[0:32, 0:hl],
                                            prod1[0:32, 0:hl],
                                            axis=AX.X, op=ALU.add)
                    nc.vector.tensor_reduce(qe1[0:32, hl:L],
                                            prod1[0:32, hl:L],
                                            axis=AX.X, op=ALU.add)
                    ew1 = st.tile([128, L], F32, name="ew1")
                    ssp = st.tile([128, 1], F32, name="ssp")
                    nc.scalar.activation(ew1[0:32, :], qe1[0:32, :], AF.Exp,
                                         scale=1.0, accum_out=ssp[0:32, :])
                    rs1 = st.tile([128, 1], F32, name="rs1")
                    nc.vector.reciprocal(rs1[0:32, :], ssp[0:32, :])
                    qw1 = st.tile([128, L], BF16, name="qw1")
                    nc.vector.tensor_scalar_mul(qw1[0:32, :], ew1[0:32, :],
                                                rs1[0:32, :])
                    qw_pk = st.tile([128, 5], BF16, name="qw_pk")
                    for g4 in range(4):
                        nc.vector.tensor_copy(qw_pk[32 * g4:32 * (g4 + 1), :],
                                              qw1[0:32, g4:L:4])

                    # --- qc flipped diag: per-l strips with 1/ssum folded
                    # in; alternate DVE/Pool so strips land just ahead of
                    # the consuming matmuls ---
                    l0s = [l for l in range(L) if n0[l] > 0]
                    c1s = [c for c in range(5) if n1[4 * c] > 0]
                    dg = wk.tile([128, L, 128], BF16, tag="diag", bufs=1,
                                 name="dg")
                    qw0 = st.tile([128, L], F32, name="qw0")
                    qw0b = st.tile([128, L], BF16, name="qw0b")
                    nc.vector.tensor_scalar_mul(qw0[:, 0:hl], ew0[:, 0:hl],
                                                rs0[:, 0:1])
                    nc.vector.tensor_scalar_mul(qw0[:, hl:L], ew0[:, hl:L],
                                                rs0[:, 0:1])
                    nc.vector.tensor_copy(qw0b[:, :], qw0[:, :])
                    for i, l in enumerate(l0s):
                        if i % 2 == 0:
                            nc.vector.tensor_scalar_mul(dg[:, l, 0:n0[l]],
                                                        ident_bf[:, 0:n0[l]],
                                                        qw0[:, l:l + 1])
                        else:
                            nc.gpsimd.tensor_mul(
                                dg[:, l:l + 1, 0:n0[l]],
                                bcast_in(qw0b[:, l:l + 1], n0[l]),
                                bcast_mid(ident_bf[:, 0:n0[l]], 1))
                    dg1 = wk.tile([128, 5, 32], BF16, tag="dg1", name="dg1")
                    sid_b = bass.AP(tensor=sid4.tensor,
                                    offset=sid4[:, :].offset,
                                    ap=[sid4[:, :].ap[0], [0, 5],
                                        sid4[:, :].ap[1]])
                    nc.gpsimd.tensor_mul(dg1[:, :, :],
                                         bcast_in(qw_pk[:, :], 32), sid_b)
                    qcT = st.tile([128, 4, BS], BF16, tag="qcT", name="qcT")
                    psqc = pq.tile([128, 512], F32, tag="qc", name="psqc")
                    for hc in range(4):
                        po = psqc[:, hc * 128:(hc + 1) * 128]
                        for i, l in enumerate(l0s):
                            nc.tensor.matmul(
                                po[:, 0:n0[l]],
                                qv_b0[:, l, hc * 128:(hc + 1) * 128],
                                dg[:, l, 0:n0[l]], start=(i == 0),
                                stop=(i == len(l0s) - 1))
                    for hc in range(4):
                        p1 = att[:, 260 + hc * 32:260 + (hc + 1) * 32]
                        for i, c in enumerate(c1s):
                            nc.tensor.matmul(
                                p1[:, 0:n1[4 * c]],
                                qv_p1[:, c, hc * 128:(hc + 1) * 128],
                                dg1[:, c, 0:n1[4 * c]], start=(i == 0),
                                stop=(i == len(c1s) - 1))
                    ps4 = psqc[:, :]
                    src0 = bass.AP(tensor=ps4.tensor, offset=ps4.offset,
                                   ap=[ps4.ap[0], [128, 4], [1, 128]])
                    nc.vector.tensor_copy(qcT[:, :, 0:128], src0)
                    pa = att[:, 260:388]
                    src1 = bass.AP(tensor=pa.tensor, offset=pa.offset,
                                   ap=[pa.ap[0], [32, 4], [1, 32]])
                    nc.scalar.copy(qcT[:, :, 128:BS], src1)

                    # --- image softmax: exp straight off PSUM, then a
                    # multiplicative 0/1 mask fused with the masked-sum ---
                    for bt in range(2):
                        pb, off = PBS[bt], BOFF[bt]
                        psi = psi2[:, bt * IL:(bt + 1) * IL]
                        ewi = st.tile([128, IL], BF16, tag="ewi", name="ewi")
                        nc.scalar.activation(ewi[:pb], psi[:pb], AF.Exp,
                                             scale=1.0)
                        ewm = st.tile([128, IL], BF16, tag="ewm", name="ewm")
                        ssi = st.tile([128, 1], F32, name="ssi")
                        nc.vector.tensor_mul(ewm[:pb], ewi[:pb],
                                             s_iem[:pb, bt, :])
                        nc.vector.tensor_reduce(ssi[:pb], ewm[:pb],
                                                axis=AX.X, op=ALU.add)
                        rsi = st.tile([128, 1], F32, name="rsi")
                        nc.vector.reciprocal(rsi[:pb], ssi[:pb])
                        iwb = st.tile([128, IL], BF16, name="iwb")
                        nc.vector.tensor_scalar_mul(iwb[:pb], ewm[:pb],
                                                    rsi[:pb])
                        iwbs.append(iwb)
                    # xt transposes for step t+2 (gathers landed last step)
                    if t + 2 < L:
                        dxts.append(fetch_tr(dgaths.pop(t + 2)))
                    for bt in range(2):
                        pb, off = PBS[bt], BOFF[bt]
                        for c in range(2):
                            tr(iwT[:, c, off:off + pb],
                               iwbs[bt][:pb, c * 128:(c + 1) * 128], pb, 128,
                               eng=nc.scalar)

                    # --- GRU ---
                    if t < L:
                        xt = dxts[t]
                        xk = [xt[:, 0, :], xt[:, 1, :], xt[:, 2, :]]
                    else:
                        xk = [dec20[:, 0, :], dec20[:, 1, :], dec20[:, 2, :]]
                    gru_t(xk, s_gi, s_gh, s_bnrow,
                          [(s_gi, qcT, 3, 4), (wivT, iwT, 0, 2)],
                          x_last=(t >= L), bns=(gbn1, gbn2, gbs))
                    # out-proj emitted here so its ctx-part matmuls fill PE
                    # during the gru elementwise tail
                    out_proj(t, qcT, iwT)
                    if t == MAX_LEN - 2:
                        argmax_reembed()

    nc.compile()
    return nc


_NC_CACHE = {}


def _get_nc(n_enc=None):
    key = tuple(n_enc) if n_enc is not None else None
    if key not in _NC_CACHE:
        _NC_CACHE[key] = build_nc(n_enc)
    return _NC_CACHE[key]


def _pad_tiles(a, ntiles):
    rows, cols = a.shape
    out = np.zeros((128 * ntiles, cols), a.dtype)
    out[:rows] = a
    return np.ascontiguousarray(
        out.reshape(ntiles, 128, cols).transpose(1, 0, 2))


def _col128(v, ncols):
    out = np.zeros((128, ncols), np.float32)
    for j in range(ncols):
        seg = v[j * 128:(j + 1) * 128]
        out[:len(seg), j] = seg
    return out


def _prep_shared(inputs):
    bf = np.float16
    f32 = np.float32
    eW = np.asarray(inputs["embed_W"], f32)
    d = {}
    # ---- decoder GRU ----
    wih = np.asarray(inputs["dec_W_ih"], f32)       # [3H, D+2H]
    whh = np.asarray(inputs["dec_W_hh"], f32)       # [3H, H]
    bih = np.asarray(inputs["dec_b_ih"], f32)
    bhh = np.asarray(inputs["dec_b_hh"], f32)
    qvb = np.asarray(inputs["qv_b"], f32)
    ivb = np.asarray(inputs["iv_b"], f32)
    fold = wih[:, D:D + H] @ qvb + wih[:, D + H:] @ ivb   # [3H]
    gi = np.zeros((128 * 11, 3 * H), f32)
    gi[0:D] = wih[:, 0:D].T
    # row 300 rides the const-1.0 row of dec_in: r/z biases (the 0.5
    # act scale halves them back), n bias unscaled
    gi[320, :2 * H] = bih[:2 * H] + bhh[:2 * H] + fold[:2 * H]
    gi[320, 2 * H:] = bih[2 * H:] + fold[2 * H:]
    gi[384:384 + H] = wih[:, D:D + H].T
    gi[896:896 + H] = wih[:, D + H:].T
    d["w_gi"] = _pad_tiles(gi.astype(bf), 11)
    gh = whh.T.copy()                                # [H, 3H]
    gh[:, 2 * H:] *= 0.5
    d["w_gh"] = _pad_tiles(gh.astype(bf), 4)
    d["bnrow"] = np.ascontiguousarray(
        (0.5 * bhh[2 * H:]).astype(bf)[None, :])
    # ---- encoder GRU ----
    ewih = np.asarray(inputs["enc_W_ih"], f32)
    ewhh = np.asarray(inputs["enc_W_hh"], f32)
    ebih = np.asarray(inputs["enc_b_ih"], f32)
    ebhh = np.asarray(inputs["enc_b_hh"], f32)
    egi = np.zeros((128 * 3, 3 * H), f32)
    egi[0:D] = ewih[:, :D].T
    egi[320, :2 * H] = ebih[:2 * H] + ebhh[:2 * H]
    egi[320, 2 * H:] = ebih[2 * H:]
    d["w_egi"] = _pad_tiles(egi.astype(bf), 3)
    egh = ewhh.T.copy()
    egh[:, 2 * H:] *= 0.5
    d["w_egh"] = _pad_tiles(egh.astype(bf), 4)
    d["ebnrow"] = np.ascontiguousarray(
        (0.5 * ebhh[2 * H:]).astype(bf)[None, :])
    # ---- output projection ----
    outW = np.asarray(inputs["out_W"], f32)          # [D, 3H]
    outb = np.asarray(inputs["out_b"], f32)
    d["w_out"] = _pad_tiles(outW.T.astype(bf), 12)
    ofold = outb + outW[:, H:2 * H] @ qvb + outW[:, 2 * H:] @ ivb
    d["outb_c"] = _col128(ofold, 3)
    # ---- attention projections ----
    d["w_qk"] = _pad_tiles(np.asarray(inputs["qk_W"], f32).T.astype(bf), 4)
    d["qkb"] = np.ascontiguousarray(
        np.asarray(inputs["qk_b"], f32).astype(bf)[None, :])
    d["w_qv"] = _pad_tiles(np.asarray(inputs["qv_W"], f32).T.astype(bf), 4)
    d["w_ak"] = _pad_tiles(np.asarray(inputs["ak_W"], f32).T.astype(bf), 4)
    d["akb"] = np.ascontiguousarray(
        np.asarray(inputs["ak_b"], f32).astype(bf)[None, :])
    akc = np.zeros((128, 1), f32)
    akc[:K, 0] = np.asarray(inputs["ak_b"], f32)
    d["akb_c"] = akc
    d["w_ik"] = _pad_tiles(np.asarray(inputs["ik_W"], f32).T.astype(bf), 2)
    ikb = np.zeros((128, 1), f32)
    ikb[:K, 0] = np.asarray(inputs["ik_b"], f32)
    d["ikb_c"] = ikb
    d["w_iv"] = _pad_tiles(np.asarray(inputs["iv_W"], f32).T.astype(bf), 2)
    # ---- embeddings ----
    emb_pad = np.zeros((V, 384), f32)
    emb_pad[:, :D] = eW
    emb_pad[:, 320] = 1.0  # gate-bias rider row
    d["emb_bf"] = emb_pad.astype(bf)
    d["qoff"] = np.broadcast_to(
        (np.arange(144, dtype=f32) // 8).astype(f32) * 512.0,
        (128, 144)).copy()
    wd_b = np.asarray(inputs["wd_b"], f32)
    aug = np.zeros((128 * 3, VP), f32)
    aug[:D, :V] = eW.T
    aug[320, :V] = wd_b
    aug[320, V:] = -60000.0  # pad columns always lose the argmax
    d["embt_bf"] = _pad_tiles(aug.astype(bf), 3)
    return d


def _idx_cols(seq_rows):
    out = np.zeros((128, 2 * L), np.uint32)
    for t in range(L):
        out[:, 2 * t] = seq_rows[0:128, t]
        out[:32, 2 * t + 1] = seq_rows[128:160, t]
    return out


def _build_maps(inputs, shared):
    """Per-core input maps. Rows within each core are sorted by question
    length (descending) so encoder steps and qc accumulation can run on
    shrinking row prefixes. Returns (maps, perms, n_enc)."""
    f32 = np.float32
    bf = np.float16
    ques = np.asarray(inputs["ques_seqs"]).astype(np.uint32)
    ans = np.asarray(inputs["ans_seqs"]).astype(np.uint32)
    qlens = np.asarray(inputs["ques_lens"]).astype(np.int64)
    img = np.asarray(inputs["img_seqs"], f32)
    maps = []
    perms = []
    counts = np.zeros((NCORES, L), np.int64)
    for s in range(NCORES):
        m = dict(shared)
        r0 = s * BS
        lens_c = qlens[r0:r0 + BS]
        perm = np.argsort(-lens_c, kind="stable")
        perms.append(perm)
        rows = r0 + perm
        lens = lens_c[perm]
        counts[s] = (lens[None, :] > np.arange(L)[:, None]).sum(axis=1)
        m["q_idx"] = _idx_cols(ques[rows][:, :L])
        m["a_idx"] = _idx_cols(ans[rows][:, :L])
        qm = np.full((128, 2, L), NEG, f32)
        for bt, (pb, off) in enumerate(zip(PBS, BOFF)):
            for b in range(pb):
                qm[b, bt, :lens[off + b]] = 0.0
        m["qmask"] = qm.astype(bf)
        qm1 = np.full((128, 5), NEG, f32)
        for g4 in range(4):
            for r in range(32):
                for c in range(5):
                    if 4 * c + g4 < lens[128 + r]:
                        qm1[32 * g4 + r, c] = 0.0
        m["qmask1"] = qm1.astype(bf)
        im = np.zeros((128, 2, IL), f32)
        for bt, (pb, off) in enumerate(zip(PBS, BOFF)):
            for b in range(pb):
                gimg = perm[off + b] // ROUNDS
                im[b, bt, gimg * 16:(gimg + 1) * 16] = 1.0
        m["ie_mask"] = im.astype(bf)
        imgs = img[s * 16:(s + 1) * 16].reshape(IL, 256)
        it = np.zeros((128 * 2, IL), f32)
        it[:256] = imgs.T
        m["img_t"] = np.ascontiguousarray(
            it.reshape(2, 128, IL).transpose(1, 0, 2)).astype(bf)
        maps.append(m)
    n_enc = [int(x) for x in counts.max(axis=0)]
    return maps, perms, n_enc


def _prepare(inputs):
    shared = _prep_shared(inputs)
    maps, perms, n_enc = _build_maps(inputs, shared)
    nc = _get_nc(n_enc)
    return nc, maps, perms


def kernel(**inputs):
    nc, in_maps, perms = _prepare(inputs)
    from concourse.bass_utils import run_bass_kernel_spmd
    res = run_bass_kernel_spmd(nc, in_maps, core_ids=list(range(NCORES)))
    outs = []
    for s in range(NCORES):
        o = np.asarray(res.results[s]["out_o"])  # [21, 3, 128, BS]
        o2 = np.concatenate([o[:, 0], o[:, 1], o[:, 2][:, :44]], axis=1)
        o2 = np.ascontiguousarray(o2.transpose(2, 0, 1))  # [BS, 21, D]
        un = np.empty_like(o2)
        un[perms[s]] = o2
        outs.append(un)
    return np.concatenate(outs, 0).astype(np.float32)

